# revision 5
# baseline (speedup 1.0000x reference)
"""Trainium2 Bass kernel v2: multi-head self-attention with RoPE (causal).

Sharding: 8 cores = 2 batches x 4 head-groups (4 heads per core), host sums
4 partial output projections per batch.

v2 design (vs baseline):
- Q/K projections computed TRANSPOSED (lhsT=W slice, rhs=xT tile) so Q^T/K^T
  land directly in [dk, s] layout -- no PE transposes for Q/K.
- RoPE applied in transposed layout: per-head column permutation
  [e0..15, o0..15, e16..31, o16..31] puts rotation partners 16 partitions
  apart within a 32-partition quadrant, so rope = mul(cos) + mul(+-sin) +
  stream_shuffle(+16 mod 32) + add, all full-width DVE ops.
- Whole kernel is software-pipelined over 512-query chunks: QKV(qc+1) and
  out-projection(qc-1..) are interleaved into attention(qc) so the PE never
  idles (keeps the HAM clock gate at 2.4 GHz) and the Scalar engine's exp
  stream (the single largest serial cost) hides under PE work.
- Scores+exp sweep writes exp(scores) to an SBUF et buffer; attn@V runs per
  query-tile afterwards, so only 2 PSUM banks hold attention accumulators.
  PSUM: 2x2 (scores, per head-pair) + 2 (attn acc) + 2 (general) = 8 banks.
- exp folds the 1/sqrt(dk) scale; softmax denominator via the appended
  ones-column of V (65-wide attn@V matmuls).
- Output partials in bf16 (summed f32 on host); PE warm-up matmuls and a
  dummy exp during the initial DMA ramp.
"""

import sys

for _p in ("/opt/trn_rl_repo",):
    if _p not in sys.path:
        sys.path.insert(0, _p)

from contextlib import ExitStack

import numpy as np

import concourse.bass as bass
import concourse.mybir as mybir
from concourse import bacc
from concourse.masks import make_identity
from concourse.tile import TileContext

B, S, D = 2, 2048, 1024
H, DK = 16, 64
NCORES = 8
CPB = NCORES // B  # cores per batch = 4
HPC = H // CPB  # heads per core = 4
HD = HPC * DK  # 256 output dims per core per projection
THETA = 10000.0

ST = 128  # sequence tile
NST = S // ST  # 16
KTD = 128  # contraction tile over model dim
NKT = D // KTD  # 8
QC = 512  # query chunk
NQC = S // QC  # 4
QTPC = QC // ST  # 4 query tiles per chunk

F32 = mybir.dt.float32
BF16 = mybir.dt.bfloat16

SHUF_MASK = [(i + 16) % 32 for i in range(32)]


def build_nc():
    nc = bacc.Bacc(
        "TRN2", target_bir_lowering=False, debug=False, num_devices=NCORES
    )
    xT = nc.dram_tensor("xT", [D, S], BF16, kind="ExternalInput").ap()
    wqT = nc.dram_tensor("wqT", [D, HD], BF16, kind="ExternalInput").ap()
    wkT = nc.dram_tensor("wkT", [D, HD], BF16, kind="ExternalInput").ap()
    wvT = nc.dram_tensor("wvT", [D, HD], BF16, kind="ExternalInput").ap()
    woT = nc.dram_tensor("woT", [HD, D], BF16, kind="ExternalInput").ap()
    cc = nc.dram_tensor("cc", [128, S], BF16, kind="ExternalInput").ap()
    ss = nc.dram_tensor("ss", [128, S], BF16, kind="ExternalInput").ap()
    part = nc.dram_tensor("part", [S, D], BF16, kind="ExternalOutput").ap()

    with TileContext(nc) as tc:
        _body(tc, xT, wqT, wkT, wvT, woT, cc, ss, part)
    nc.compile()
    return nc


def _body(tc, xT, wqT, wkT, wvT, woT, cc, ss, part):
    nc = tc.nc
    with ExitStack() as ctx:
        consts = ctx.enter_context(tc.tile_pool(name="consts", bufs=1))

        wq_sb = consts.tile([128, NKT, HD], BF16)
        wk_sb = consts.tile([128, NKT, HD], BF16)
        wv_sb = consts.tile([128, NKT, HD], BF16)
        wo_sb = consts.tile([128, HD // 128, D], BF16)
        cc_sb = consts.tile([128, NQC, QC], BF16)
        ss_sb = consts.tile([128, NQC, QC], BF16)
        identb = consts.tile([128, 128], BF16)
        v65 = consts.tile([128, NST, HPC * 65], BF16)
        qta = consts.tile([128, NST, ST], BF16)
        qtb = consts.tile([128, NST, ST], BF16)
        kta = consts.tile([128, NST, ST], BF16)
        ktb = consts.tile([128, NST, ST], BF16)
        # exp(scores) ring buffer: [slot, hp, hl, q]. 20 slots with cumulative
        # per-chunk bases so exp of chunk qc never overwrites slots attn@V of
        # chunk qc-1 is still reading (keeps the scalar exp stream gapless
        # across chunk boundaries).
        ETS = 20
        etbuf = consts.tile([128, ETS, 2, 2, QC], BF16)
        ET_BASE = [0, 4, 12, 4]  # cumulative mod 20

        def et_slot(qc, kt):
            return (ET_BASE[qc] + kt) % ETS
        anat = consts.tile([128, NST, HPC, DK], BF16)
        outta = consts.tile([128, NST, ST], BF16)
        outtb = consts.tile([128, NST, ST], BF16)
        warm = consts.tile([128, QC], BF16)
        dummy = consts.tile([128, 16], F32)

        qta_f = qta.rearrange("p a b -> p (a b)")
        qtb_f = qtb.rearrange("p a b -> p (a b)")
        kta_f = kta.rearrange("p a b -> p (a b)")
        ktb_f = ktb.rearrange("p a b -> p (a b)")
        outta_f = outta.rearrange("p a b -> p (a b)")
        outtb_f = outtb.rearrange("p a b -> p (a b)")

        with (
            tc.tile_pool(name="px", bufs=2) as px,
            tc.tile_pool(name="pt", bufs=1) as ptool,
            tc.tile_pool(name="pog", bufs=2) as pog,
            tc.tile_pool(name="pr", bufs=2) as pr,
            tc.tile_pool(name="pps", bufs=1, space="PSUM") as pps,
            tc.tile_pool(name="pacc", bufs=2, space="PSUM") as pacc,
            tc.tile_pool(name="pgen", bufs=2, space="PSUM") as pgen,
        ):
            xt_tiles = {}

            def dma_x(qc):
                # two half-column DMAs so compute on the first 256 columns
                # can start while the second half is still in flight
                xt = px.tile([128, NKT, QC], BF16, tag="xt", name=f"xt{qc}")
                for h in range(2):
                    nc.sync.dma_start(
                        xt[:, :, h * 256 : (h + 1) * 256],
                        xT[
                            :, qc * QC + h * 256 : qc * QC + (h + 1) * 256
                        ].rearrange("(kt p) s -> p kt s", p=128),
                    )
                xt_tiles[qc] = xt

            # DMA priority order: everything chunk-0 compute needs first.
            nc.sync.dma_start(
                wq_sb[:], wqT.rearrange("(kt p) h -> p kt h", p=128)
            )
            dma_x(0)
            nc.sync.dma_start(
                wk_sb[:], wkT.rearrange("(kt p) h -> p kt h", p=128)
            )
            nc.sync.dma_start(cc_sb[:, 0, :], cc[:, 0:QC])
            nc.sync.dma_start(ss_sb[:, 0, :], ss[:, 0:QC])
            nc.sync.dma_start(
                wv_sb[:], wvT.rearrange("(kt p) h -> p kt h", p=128)
            )
            for qcb in range(1, NQC):
                nc.sync.dma_start(
                    cc_sb[:, qcb, :], cc[:, qcb * QC : (qcb + 1) * QC]
                )
                nc.sync.dma_start(
                    ss_sb[:, qcb, :], ss[:, qcb * QC : (qcb + 1) * QC]
                )
            nc.sync.dma_start(wo_sb[:], woT.rearrange("(i p) o -> p i o", p=128))
            make_identity(nc, identb[:])
            # only the ones-columns need initializing; the V scatter writes
            # the rest
            nc.vector.memset(warm[:], 0.125)
            nc.vector.memset(
                v65.rearrange("p st (h e) -> p st h e", h=HPC)[:, :, :, DK : DK + 1],
                1.0,
            )

            def unit_warm(n):
                # keep the PE busy during the initial DMA ramp so the HAM
                # clock gate opens before real work, and preload the ACT
                # exp table.
                for i in range(n):
                    ps = pps.tile(
                        [128, 2, QC], F32, tag=f"pst{i % 2}", name=f"warmps{i}"
                    )
                    nc.tensor.matmul(
                        ps[:, 0, :],
                        lhsT=warm[:, 0:128],
                        rhs=warm[:],
                        start=True,
                        stop=True,
                    )
                    if i == 0:
                        nc.scalar.activation(
                            out=dummy[:],
                            in_=ps[:, 0, 0:16],
                            func=mybir.ActivationFunctionType.Exp,
                            scale=1.0,
                        )

            def unit_qkT_mm(qc, which, g, state, half):
                # one half (4 kt steps) of a Q/K projection group; halves are
                # adjacent filler entries (no other pgen user in between) but
                # score steps can be paced between them
                w_sb = wq_sb if which == "q" else wk_sb
                xt = xt_tiles[qc]
                if half == 0:
                    state["ps"] = pgen.tile(
                        [128, QC], F32, tag="gen", name=f"ps{which}{qc}{g}"
                    )
                ps = state["ps"]
                for kt in range(half * 4, half * 4 + 4):
                    if qc == 0:
                        # chunk 0: x arrives in two half-column DMAs; split
                        # the moving so compute starts on the first half
                        for h in range(2):
                            nc.tensor.matmul(
                                ps[:, h * 256 : (h + 1) * 256],
                                lhsT=w_sb[:, kt, g * 128 : (g + 1) * 128],
                                rhs=xt[:, kt, h * 256 : (h + 1) * 256],
                                start=(kt == 0 and h == 0),
                                stop=(kt == NKT - 1 and h == 1),
                            )
                    else:
                        nc.tensor.matmul(
                            ps[:],
                            lhsT=w_sb[:, kt, g * 128 : (g + 1) * 128],
                            rhs=xt[:, kt, :],
                            start=(kt == 0),
                            stop=(kt == NKT - 1),
                        )
                if half == 1:
                    _qkT_rope(qc, which, g, ps)

            def unit_qkT(qc, which, g):
                state = {}
                unit_qkT_mm(qc, which, g, state, 0)
                unit_qkT_mm(qc, which, g, state, 1)

            def _qkT_rope(qc, which, g, ps):
                dst = {
                    ("q", 0): qta_f,
                    ("q", 1): qtb_f,
                    ("k", 0): kta_f,
                    ("k", 1): ktb_f,
                }[(which, g)]
                t1 = ptool.tile([128, QC], BF16, tag="t1", name=f"t1{which}{qc}{g}")
                t2 = ptool.tile([128, QC], BF16, tag="t2", name=f"t2{which}{qc}{g}")
                t2s = ptool.tile([128, QC], BF16, tag="t2s", name=f"t2s{which}{qc}{g}")
                if qc == 0:
                    # chunk 0: Scalar is idle, let it downcast the projection
                    # so the DVE rope muls run in 2x bf16 mode
                    psb = ptool.tile(
                        [128, QC], BF16, tag="psb", name=f"psb{which}{g}"
                    )
                    nc.scalar.copy(psb[:], ps[:])
                    nc.vector.tensor_mul(t1[:], psb[:], cc_sb[:, qc, :])
                    nc.vector.tensor_mul(t2[:], psb[:], ss_sb[:, qc, :])
                else:
                    nc.vector.tensor_mul(t1[:], ps[:], cc_sb[:, qc, :])
                    nc.vector.tensor_mul(t2[:], ps[:], ss_sb[:, qc, :])
                nc.vector.stream_shuffle(t2s[:], t2[:], SHUF_MASK)
                nc.vector.tensor_add(
                    dst[:, qc * QC : (qc + 1) * QC], t1[:], t2s[:]
                )

            def unit_v(qc, stl):
                st = qc * QTPC + stl
                xt = xt_tiles[qc]
                ps = pgen.tile([128, HD], F32, tag="gen", name=f"psv{st}")
                for kt in range(NKT):
                    nc.tensor.matmul(
                        ps[:],
                        lhsT=xt[:, kt, stl * ST : (stl + 1) * ST],
                        rhs=wv_sb[:, kt, :],
                        start=(kt == 0),
                        stop=(kt == NKT - 1),
                    )
                eng = nc.scalar.copy if qc == 0 else nc.vector.tensor_copy
                eng(
                    v65[:, st, :].rearrange("p (h e) -> p h e", h=HPC)[:, :, 0:DK],
                    ps[:].rearrange("p (h e) -> p h e", h=HPC),
                )

            def unit_scores(qc, kt):
                c0 = max(0, kt * ST - qc * QC)
                for hp, (qt_, kt_) in enumerate(
                    ((qta_f, kta_f), (qtb_f, ktb_f))
                ):
                    pst = pps.tile(
                        [128, 2, QC], F32, tag=f"pst{hp}", name=f"pst{qc}_{kt}_{hp}"
                    )
                    for hl in range(2):
                        po = 64 * hl
                        nc.tensor.matmul(
                            pst[:, hl, c0:QC],
                            lhsT=kt_[po : po + 64, kt * ST : (kt + 1) * ST],
                            rhs=qt_[po : po + 64, qc * QC + c0 : (qc + 1) * QC],
                            start=True,
                            stop=True,
                        )
                    nc.scalar.activation(
                        out=etbuf[:, et_slot(qc, kt), hp, :, c0:QC],
                        in_=pst[:, :, c0:QC],
                        func=mybir.ActivationFunctionType.Exp,
                        scale=1.0 / (DK**0.5),
                    )
                    if c0 > 0 or kt * ST == qc * QC:
                        # diagonal block: zero the strictly-upper part
                        for hl in range(2):
                            nc.gpsimd.affine_select(
                                out=etbuf[:, et_slot(qc, kt), hp, hl, c0 : c0 + ST],
                                in_=etbuf[:, et_slot(qc, kt), hp, hl, c0 : c0 + ST],
                                compare_op=mybir.AluOpType.is_ge,
                                fill=0.0,
                                base=0,
                                pattern=[[1, ST]],
                                channel_multiplier=-1,
                            )

            def unit_av(qc, qtl):
                qt = qc * QTPC + qtl
                pa = pacc.tile([128, 512], F32, tag="pattn", name=f"pattn{qt}")
                for kt in range(qt + 1):
                    for hp in range(2):
                        for hl in range(2):
                            h = hp * 2 + hl
                            nc.tensor.matmul(
                                pa[:, h * 65 : h * 65 + 65],
                                lhsT=etbuf[
                                    :, et_slot(qc, kt), hp, hl, qtl * ST : (qtl + 1) * ST
                                ],
                                rhs=v65[:, kt, h * 65 : h * 65 + 65],
                                start=(kt == 0 and h == 0),
                                stop=(kt == qt and h == HPC - 1),
                            )
                rt = pr.tile([128, HPC], F32, tag="recip", name=f"rt{qt}")
                nc.vector.reciprocal(
                    rt[:],
                    bass.AP(pa.tensor, pa.offset + 64, [pa.ap[0], [65, HPC]]),
                )
                for h in range(HPC):
                    nc.vector.tensor_scalar_mul(
                        anat[:, qt, h, :],
                        pa[:, h * 65 : h * 65 + 64],
                        rt[:, h : h + 1],
                    )

            def unit_out(qc, stl, evac, rhsplit=2):
                st = qc * QTPC + stl
                for hp, outt in ((0, outta), (1, outtb)):
                    pt = pgen.tile([128, ST], BF16, tag="gen", name=f"pt{st}{hp}")
                    nc.tensor.transpose(
                        pt[:],
                        anat[:, st, hp * 2 : hp * 2 + 2, :],
                        identb[:],
                    )
                    nc.vector.tensor_copy(outt[:, st, :], pt[:])
                og = pog.tile([128, 2, 512], BF16, tag="og", name=f"og{st}")
                for oc in range(2):
                    po = pgen.tile([128, 512], F32, tag="gen", name=f"po{st}{oc}")
                    for i, of in enumerate((outta_f, outtb_f)):
                        nc.tensor.matmul(
                            po[:],
                            lhsT=of[:, st * ST : (st + 1) * ST],
                            rhs=wo_sb[:, i, oc * 512 : (oc + 1) * 512],
                            start=(i == 0),
                            stop=(i == 1),
                        )
                    if evac == "v" or (evac == "vs" and oc == 0):
                        nc.vector.tensor_copy(og[:, oc, :], po[:])
                    else:
                        nc.scalar.copy(og[:, oc, :], po[:])
                # full 2KB-row DMAs (both oc halves) -- half the descriptor
                # count of per-oc transfers; row-split across queues
                rs = ST // rhsplit
                ogf = og.rearrange("p a b -> p (a b)")
                for rh in range(rhsplit):
                    nc.sync.dma_start(
                        part[st * ST + rh * rs : st * ST + (rh + 1) * rs, :],
                        ogf[rh * rs : (rh + 1) * rs, :],
                    )

            # ---- window 1: warmup + QKV chunk 0 ----
            unit_warm(14)
            for g in (0, 1):
                unit_qkT(0, "q", g)
            for g in (0, 1):
                unit_qkT(0, "k", g)
            for stl in range(QTPC):
                unit_v(0, stl)

            # ---- windows 2-5: attention(qc) with interleaved fillers ----
            for qc in range(NQC):
                fillers = []
                if qc + 1 < NQC:
                    fillers.append((0.05, lambda qc=qc: dma_x(qc + 1)))
                    for which in ("q", "k"):
                        for g in (0, 1):
                            st8 = {}
                            for half in (0, 1):
                                fillers.append(
                                    (
                                        0.95,
                                        lambda qc=qc, w=which, g=g, s=st8, h=half: unit_qkT_mm(
                                            qc + 1, w, g, s, h
                                        ),
                                    )
                                )
                    if qc + 1 < NQC - 1:
                        for stl in range(QTPC):
                            fillers.append(
                                (0.95, lambda qc=qc, s=stl: unit_v(qc + 1, s))
                            )
                    else:
                        # last QKV window: defer V(3) to the next window and
                        # start chunk-0's out-projection instead, so its
                        # output DMA gets runway
                        for stl in range(QTPC):
                            fillers.append(
                                (1.3, lambda s=stl: unit_out(0, s, evac="v"))
                            )
                else:
                    for stl in range(QTPC):
                        fillers.append(
                            (0.95, lambda s=stl: unit_v(NQC - 1, s))
                        )
                    for p in range(1, NQC - 1):
                        for stl in range(QTPC):
                            fillers.append(
                                (1.3, lambda p=p, s=stl: unit_out(p, s, evac="v"))
                            )

                T = 4 * qc + 4
                start_kt = 0 if qc == 0 else 2  # 0,1 pre-emitted last window
                last = qc == NQC - 1
                nsteps = (T - start_kt) + (2 + QTPC if not last else 0)
                total_dur = sum(d for d, _ in fillers)
                emitted = 0
                edur = 0.0
                step = 0
                for kt in range(start_kt, T):
                    unit_scores(qc, kt)
                    if last and kt >= T - QTPC:
                        # last chunk: fold av/out into the diagonal of the
                        # score sweep so the PE stays dense to the end. ACT
                        # is still streaming exp here, so og evacs go on DVE.
                        qtl = kt - (T - QTPC)
                        unit_av(qc, qtl)
                        if qtl >= 1:
                            unit_out(qc, qtl - 1, evac="v")
                    step += 1
                    want = step / nsteps * total_dur
                    while emitted < len(fillers) and edur < want - 1e-9:
                        d, fn = fillers[emitted]
                        fn()
                        edur += d
                        emitted += 1
                if not last:
                    # pre-emit the next chunk's first score iterations so the
                    # scalar exp stream stays gapless across the av sweep
                    while emitted < len(fillers):
                        fillers[emitted][1]()
                        emitted += 1
                    unit_scores(qc + 1, 0)
                    unit_scores(qc + 1, 1)
                    for qtl in range(QTPC):
                        unit_av(qc, qtl)
                        step += 1
                        want = step / nsteps * total_dur
                        while emitted < len(fillers) and edur < want - 1e-9:
                            d, fn = fillers[emitted]
                            fn()
                            edur += d
                            emitted += 1
                else:
                    while emitted < len(fillers):
                        fillers[emitted][1]()
                        emitted += 1

            # ---- tail: final out-projection tile ----
            unit_out(NQC - 1, QTPC - 1, evac="vs", rhsplit=4)


_NC_CACHE = None


def _get_nc():
    global _NC_CACHE
    if _NC_CACHE is None:
        _NC_CACHE = build_nc()
    return _NC_CACHE


def prep_in_maps(x, token_positions, Wq, Wk, Wv, Wo):
    x = np.asarray(x, dtype=np.float32)
    pos = np.asarray(token_positions)
    Wq = np.asarray(Wq, dtype=np.float32)
    Wk = np.asarray(Wk, dtype=np.float32)
    Wv = np.asarray(Wv, dtype=np.float32)
    Wo = np.asarray(Wo, dtype=np.float32)

    # per-head output-dim permutation: [e0..15, o0..15, e16..31, o16..31]
    perm = np.concatenate(
        [
            np.arange(0, 32, 2),
            np.arange(1, 32, 2),
            np.arange(32, 64, 2),
            np.arange(33, 64, 2),
        ]
    )

    inv_freq = (THETA ** (-(np.arange(0, DK, 2, dtype=np.float32) / DK))).astype(
        np.float32
    )

    import ml_dtypes

    bf16 = ml_dtypes.bfloat16
    xT_b = [np.ascontiguousarray(x[b].T).astype(bf16) for b in range(B)]

    # transposed-rope tables [128, S]: row p covers pair k(p), sign for the
    # +-sin table; the 2-head 128-row pattern repeats for both head pairs.
    p = np.arange(128)
    l = p % 64
    quad = l // 32
    idx = l % 32
    kk = quad * 16 + (idx % 16)
    sign = np.where(idx < 16, 1.0, -1.0).astype(np.float32)
    cc_b, ss_b = [], []
    for b in range(B):
        ang = pos[b].astype(np.float32)[None, :] * inv_freq[kk][:, None]
        cc_b.append(np.ascontiguousarray(np.cos(ang)).astype(bf16))
        ss_b.append(
            np.ascontiguousarray(sign[:, None] * np.sin(ang)).astype(bf16)
        )

    in_maps = []
    for c in range(NCORES):
        b = c // CPB
        h0 = (c % CPB) * HPC
        cols_p = np.concatenate([(h0 + i) * DK + perm for i in range(HPC)])
        cols_n = np.arange(h0 * DK, (h0 + HPC) * DK)
        in_maps.append(
            {
                "xT": xT_b[b],
                "wqT": np.ascontiguousarray(Wq.T[:, cols_p]).astype(bf16),
                "wkT": np.ascontiguousarray(Wk.T[:, cols_p]).astype(bf16),
                "wvT": np.ascontiguousarray(Wv.T[:, cols_n]).astype(bf16),
                "woT": np.ascontiguousarray(Wo.T[cols_n, :]).astype(bf16),
                "cc": cc_b[b],
                "ss": ss_b[b],
            }
        )
    return in_maps


def gather(results):
    out = np.zeros((B, S, D), dtype=np.float32)
    for c, res in enumerate(
        results.results if hasattr(results, "results") else results
    ):
        out[c // CPB] += np.asarray(res["part"], dtype=np.float32)
    return out


def kernel(x, token_positions, Wq, Wk, Wv, Wo, _trace=False, _results_box=None):
    from concourse.bass_utils import run_bass_kernel_spmd

    nc = _get_nc()
    in_maps = prep_in_maps(x, token_positions, Wq, Wk, Wv, Wo)
    res = run_bass_kernel_spmd(
        nc, in_maps, core_ids=list(range(NCORES)), trace=_trace
    )
    if _results_box is not None:
        _results_box.append(res)
    return gather(res.results)

